# revision 3
# baseline (speedup 1.0000x reference)
"""Trainium2 Bass kernel for nn_CrossAttentionPro (chained cross-attention).

Sharding: 8 cores = data-parallel over B (2) x head-parallel (4 head-pairs).
Each core computes, for one batch b and heads (2*hp, 2*hp+1):
  - shared qkv projection of x and y restricted to its heads (column-sharded
    qkv_w), with attention scales folded into q biases/scales
  - catt_x2yT[m,t] = k_y q_x^T (pre-transposed layout), catt_y2x[m,s]
    (carries the extra chained 1/8 via q_y scale 1/64)
  - softmax-free-of-max attention both stages via exp + ones-column matmul
    (appended ones column of v gives the softmax denominator for free)
  - chainedT[s,t] accumulated per 128-row s-block, exp'd, and consumed
    immediately by the cval2 matmul (flash-style, never hits HBM)
  - partial projection out_partial[t, :] = diffT.T @ proj_w.T[c_slice]
Host sums the 4 head-pair partials per batch and adds proj_b.

All heavy matmuls run in float32r (full PE rate, ~1.5e-4 matmul rel err).
"""

import math
import numpy as np

B, T, MM, C, H = 2, 2048, 1024, 512, 8
D = 64
NC = 8
NMB = MM // 128  # 8 m-blocks
NSB = T // 128  # 16 s-blocks
NTC = T // 512  # 4 t-chunks of 512
_kernels = {}


def _install_ntff_hook():
    """Bridge antenv.axon_hooks for NTFF profiling (missing in this image)."""
    import contextlib, ctypes, sys, types

    if "antenv.axon_hooks" in sys.modules:
        return
    try:
        import antenv
    except ImportError:
        return

    def _make_hook():
        try:
            lib = ctypes.CDLL("/opt/axon/libaxon_pjrt.so")
        except OSError:
            return None
        if not hasattr(lib, "axon_start_nrt_profile"):
            return None
        lib.axon_start_nrt_profile.argtypes = [
            ctypes.POINTER(ctypes.c_int64),
            ctypes.c_size_t,
        ]
        lib.axon_start_nrt_profile.restype = ctypes.c_int64
        lib.axon_stop_nrt_profile.argtypes = [ctypes.c_char_p]
        lib.axon_stop_nrt_profile.restype = ctypes.c_int64

        @contextlib.contextmanager
        def _hook(output_dir, device_ids):
            import jax

            jax.devices()
            if device_ids:
                ids = (ctypes.c_int64 * len(device_ids))(*device_ids)
                rc = lib.axon_start_nrt_profile(ids, len(device_ids))
            else:
                rc = lib.axon_start_nrt_profile(None, 0)
            if rc != 0:
                raise RuntimeError(f"axon_start_nrt_profile rc={rc}")
            try:
                yield
            finally:
                n = lib.axon_stop_nrt_profile(str(output_dir).encode())
                if n < 0:
                    raise RuntimeError(f"axon_stop_nrt_profile rc={n}")

        return _hook

    m = types.ModuleType("antenv.axon_hooks")
    m._hook = _make_hook()
    m.get_axon_ntff_profile_hook = lambda: m._hook
    m.set_axon_ntff_profile_hook = lambda h: setattr(m, "_hook", h)
    sys.modules["antenv.axon_hooks"] = m
    antenv.axon_hooks = m


def _build(use_mask):
    import concourse.bass as bass
    import concourse.mybir as mybir
    import concourse.tile as tile
    from concourse import bacc
    from concourse.bass import ts
    from concourse.masks import make_identity

    dt = mybir.dt
    AF = mybir.ActivationFunctionType
    ALU = mybir.AluOpType

    nc = bacc.Bacc("TRN2", target_bir_lowering=False, debug=False, num_devices=NC)
    xT_d = nc.dram_tensor("xT", [C, T], dt.float32r, kind="ExternalInput").ap()
    yT_d = nc.dram_tensor("yT", [C, MM], dt.float32r, kind="ExternalInput").ap()
    wT_d = nc.dram_tensor("wT", [C, 384], dt.float32r, kind="ExternalInput").ap()
    bx_d = nc.dram_tensor("bias_x", [3, 128, 1], dt.float32, kind="ExternalInput").ap()
    by_d = nc.dram_tensor("bias_y", [3, 128, 1], dt.float32, kind="ExternalInput").ap()
    pw_d = nc.dram_tensor("projT", [128, C], dt.float32r, kind="ExternalInput").ap()
    if use_mask:
        mk_d = nc.dram_tensor("mask01T", [T, T], dt.float32, kind="ExternalInput").ap()
    out_d = nc.dram_tensor("out_partial", [T, C], dt.float32, kind="ExternalOutput").ap()

    with tile.TileContext(nc) as tc:
        # pool stack (LIFO): pconst > pvaug > pdiff > pqkv > {pin,pvt} /
        # {pcatt,pE,(pmk),pnorm} / {pout}
        pconst_cm = tc.tile_pool(name="pconst", bufs=1)
        pconst = pconst_cm.__enter__()
        pvaug_cm = tc.tile_pool(name="pvaug", bufs=1)
        pvaug = pvaug_cm.__enter__()
        pdiff_cm = tc.tile_pool(name="pdiff", bufs=1)
        pdiff = pdiff_cm.__enter__()
        pqkv_cm = tc.tile_pool(name="pqkv", bufs=1)
        pqkv = pqkv_cm.__enter__()
        pin_cm = tc.tile_pool(name="pin", bufs=1)
        pin = pin_cm.__enter__()
        pvt_cm = tc.tile_pool(name="pvt", bufs=1)
        pvt = pvt_cm.__enter__()

        # ---- constants ----
        ident = pconst.tile([128, 128], dt.float32, tag="ident")
        make_identity(nc, ident[:])
        ones64 = pconst.tile([1, 64], dt.float32, tag="ones64")
        nc.vector.memset(ones64[:], 1.0)
        ones16 = pconst.tile([128, 16], dt.float32, tag="ones16")
        nc.vector.memset(ones16[:], 1.0)
        biases = {}
        for i, nm in enumerate(["q", "k", "v"]):
            bx = pconst.tile([128, 1], dt.float32, tag=f"bx_{nm}")
            nc.sync.dma_start(bx[:], bx_d[i])
            by = pconst.tile([128, 1], dt.float32, tag=f"by_{nm}")
            nc.sync.dma_start(by[:], by_d[i])
            biases[("x", nm)] = bx
            biases[("y", nm)] = by
        projT_s = pconst.tile([128, C], dt.float32r, tag="projT")
        nc.sync.dma_start(projT_s[:], pw_d[:])

        # ---- stage A: load inputs, qkv projections, v transposes ----
        with nc.named_scope("stageA"):
            xT = [pin.tile([128, T], dt.float32r, tag=f"xT{i}", name=f"xT{i}") for i in range(4)]
            yT = [pin.tile([128, MM], dt.float32r, tag=f"yT{i}", name=f"yT{i}") for i in range(4)]
            wT = [pin.tile([128, 384], dt.float32r, tag=f"wT{i}", name=f"wT{i}") for i in range(4)]
            for i in range(4):
                nc.sync.dma_start(xT[i][:], xT_d[ts(i, 128), :])
                nc.sync.dma_start(yT[i][:], yT_d[ts(i, 128), :])
                nc.sync.dma_start(wT[i][:], wT_d[ts(i, 128), :])

            qx = pqkv.tile([128, T], dt.float32r, tag="qx")
            kx = pqkv.tile([128, T], dt.float32r, tag="kx")
            qy = pqkv.tile([128, MM], dt.float32r, tag="qy")
            ky = pqkv.tile([128, MM], dt.float32r, tag="ky")
            vxT = pvt.tile([128, T], dt.float32, tag="vxT")
            vyT = pvt.tile([128, MM], dt.float32, tag="vyT")

            projs = [
                (qx, xT, T, 0, ("x", "q"), 1.0 / 8),
                (kx, xT, T, 128, ("x", "k"), 1.0),
                (vxT, xT, T, 256, ("x", "v"), 1.0),
                (qy, yT, MM, 0, ("y", "q"), 1.0 / 64),
                (ky, yT, MM, 128, ("y", "k"), 1.0),
                (vyT, yT, MM, 256, ("y", "v"), 1.0),
            ]
            psa_cm = tc.tile_pool(name="psA", bufs=2, space="PSUM")
            psa = psa_cm.__enter__()
            for out_t, src, n_t, wcol, bkey, scale in projs:
                ps = psa.tile([128, n_t], dt.float32, tag="pa")
                for c in range(4):
                    for tcj in range(n_t // 512):
                        nc.tensor.matmul(
                            ps[:, ts(tcj, 512)],
                            wT[c][:, wcol : wcol + 128],
                            src[c][:, ts(tcj, 512)],
                            start=(c == 0),
                            stop=(c == 3),
                        )
                nc.scalar.activation(
                    out_t[:], ps[:], AF.Identity, bias=biases[bkey][:], scale=scale
                )
            psa_cm.__exit__(None, None, None)

            # v transposes -> v_aug tensors (both heads), ones col appended
            vx_aug = [
                pvaug.tile([128, 65 * NSB], dt.float32r, tag=f"vx_aug{h}", name=f"vx_aug{h}")
                for h in range(2)
            ]
            vy_aug = [
                pvaug.tile([128, 65 * NMB], dt.float32r, tag=f"vy_aug{h}", name=f"vy_aug{h}")
                for h in range(2)
            ]
            pst_cm = tc.tile_pool(name="psT", bufs=4, space="PSUM")
            pst = pst_cm.__enter__()
            for src, aug, nblk in [(vxT, vx_aug, NSB), (vyT, vy_aug, NMB)]:
                for i in range(nblk):
                    tp = pst.tile([128, 128], dt.float32, tag="tp")
                    nc.tensor.transpose(tp[:], src[:, ts(i, 128)], ident[:])
                    for h in range(2):
                        nc.vector.tensor_copy(
                            aug[h][:, 65 * i : 65 * i + 64], tp[:, ts(h, 64)]
                        )
            pst_cm.__exit__(None, None, None)
            for aug, nblk in [(vx_aug, NSB), (vy_aug, NMB)]:
                for h in range(2):
                    nc.vector.tensor_copy(
                        aug[h][:, 64 : 65 * nblk : 65], ones16[:, 0:nblk]
                    )
        pvt_cm.__exit__(None, None, None)
        pin_cm.__exit__(None, None, None)

        pcatt_cm = tc.tile_pool(name="pcatt", bufs=1)
        pcatt = pcatt_cm.__enter__()
        pe_cm = tc.tile_pool(name="pE", bufs=1 if use_mask else 2)
        pE = pe_cm.__enter__()
        if use_mask:
            pmk_cm = tc.tile_pool(name="pmk", bufs=2)
            pmk = pmk_cm.__enter__()
        pnorm_cm = tc.tile_pool(name="pnorm", bufs=1)
        pnorm = pnorm_cm.__enter__()

        diffT = pdiff.tile([128, T], dt.float32r, tag="diffT")

        for h in range(2):
            hh = slice(64 * h, 64 * h + 64)
            cxT = pcatt.tile([128, T * NMB], dt.float32r, tag="cxT")
            cyx = pcatt.tile([128, T * NMB], dt.float32r, tag="cyx")

            # ---- stage B + C1: catt tiles, E1, cval1 accumulation ----
            with nc.named_scope(f"B{h}"):
                psb_cm = tc.tile_pool(name=f"psB{h}", bufs=1, space="PSUM")
                psb = psb_cm.__enter__()
                cv1 = psb.tile([65, T], dt.float32, tag="cv")
                for mb in range(NMB):
                    for half in range(2):
                        pb = psb.tile([128, 1024], dt.float32, tag="pb", bufs=2)
                        for tcj in range(2):
                            nc.tensor.matmul(
                                pb[:, ts(tcj, 512)],
                                ky[hh, ts(mb, 128)],
                                qx[hh, half * 1024 + tcj * 512 : half * 1024 + (tcj + 1) * 512],
                                start=True,
                                stop=True,
                            )
                        nc.vector.tensor_copy(
                            cxT[:, mb * T + half * 1024 : mb * T + (half + 1) * 1024],
                            pb[:],
                        )
                        e1 = pE.tile([128, 1024], dt.float32r, tag="E")
                        nc.scalar.activation(e1[:], pb[:], AF.Exp)
                        for tcj in range(2):
                            nc.tensor.matmul(
                                cv1[:, half * 1024 + tcj * 512 : half * 1024 + (tcj + 1) * 512],
                                vy_aug[h][:, 65 * mb : 65 * mb + 65],
                                e1[:, ts(tcj, 512)],
                                start=(mb == 0),
                                stop=(mb == NMB - 1),
                            )
                    for half in range(2):
                        pb2 = psb.tile([128, 1024], dt.float32, tag="pb", bufs=2)
                        for tcj in range(2):
                            nc.tensor.matmul(
                                pb2[:, ts(tcj, 512)],
                                qy[hh, ts(mb, 128)],
                                kx[hh, half * 1024 + tcj * 512 : half * 1024 + (tcj + 1) * 512],
                                start=True,
                                stop=True,
                            )
                        nc.vector.tensor_copy(
                            cyx[:, mb * T + half * 1024 : mb * T + (half + 1) * 1024],
                            pb2[:],
                        )

                # cval1 normalization: r1 = 1/sumexp; cv1n = cv1 * bcast(r1)
                r1 = pnorm.tile([1, T], dt.float32, tag="r")
                nc.vector.reciprocal(r1[:], cv1[64:65, :])
                cv1n = pnorm.tile([64, T], dt.float32, tag="cv1n")
                for half in range(2):
                    rbs = pnorm.tile([64, 1024], dt.float32, tag="rbs", bufs=2)
                    for tcj in range(2):
                        rb = psb.tile([64, 512], dt.float32, tag="pb", bufs=2)
                        nc.tensor.matmul(
                            rb[:],
                            ones64[:],
                            r1[:, half * 1024 + tcj * 512 : half * 1024 + (tcj + 1) * 512],
                            start=True,
                            stop=True,
                        )
                        nc.vector.tensor_copy(rbs[:, ts(tcj, 512)], rb[:])
                    nc.vector.tensor_tensor(
                        cv1n[:, half * 1024 : (half + 1) * 1024],
                        cv1[0:64, half * 1024 : (half + 1) * 1024],
                        rbs[:],
                        ALU.mult,
                    )
                psb_cm.__exit__(None, None, None)

            # ---- stage C2: chained scores + cval2, flash-style ----
            with nc.named_scope(f"C{h}"):
                psc_cm = tc.tile_pool(name=f"psC{h}", bufs=1, space="PSUM")
                psc = psc_cm.__enter__()
                cv2 = psc.tile([65, T], dt.float32, tag="cv")
                for sbi in range(NSB):
                    for half in range(2):
                        ch = psc.tile([128, 1024], dt.float32, tag="ch", bufs=2)
                        for mb in range(NMB):
                            for tcj in range(2):
                                nc.tensor.matmul(
                                    ch[:, ts(tcj, 512)],
                                    cyx[:, mb * T + sbi * 128 : mb * T + (sbi + 1) * 128],
                                    cxT[:, mb * T + half * 1024 + tcj * 512 : mb * T + half * 1024 + (tcj + 1) * 512],
                                    start=(mb == 0),
                                    stop=(mb == NMB - 1),
                                )
                        e2 = pE.tile([128, 1024], dt.float32r, tag="E")
                        nc.scalar.activation(e2[:], ch[:], AF.Exp)
                        if use_mask:
                            for tcj in range(2):
                                mk = pmk.tile([128, 512], dt.float32, tag="mk")
                                nc.sync.dma_start(
                                    mk[:],
                                    mk_d[
                                        ts(sbi, 128),
                                        half * 1024 + tcj * 512 : half * 1024 + (tcj + 1) * 512,
                                    ],
                                )
                                nc.vector.tensor_tensor(
                                    e2[:, ts(tcj, 512)],
                                    e2[:, ts(tcj, 512)],
                                    mk[:],
                                    ALU.mult,
                                )
                        for tcj in range(2):
                            nc.tensor.matmul(
                                cv2[:, half * 1024 + tcj * 512 : half * 1024 + (tcj + 1) * 512],
                                vx_aug[h][:, 65 * sbi : 65 * sbi + 65],
                                e2[:, ts(tcj, 512)],
                                start=(sbi == 0),
                                stop=(sbi == NSB - 1),
                            )

                # cval2 normalization + diff into diffT rows of this head
                r2 = pnorm.tile([1, T], dt.float32, tag="r")
                nc.vector.reciprocal(r2[:], cv2[64:65, :])
                for half in range(2):
                    rbs2 = pnorm.tile([64, 1024], dt.float32, tag="rbs", bufs=2)
                    for tcj in range(2):
                        rb = psc.tile([64, 512], dt.float32, tag="ch", bufs=2)
                        nc.tensor.matmul(
                            rb[:],
                            ones64[:],
                            r2[:, half * 1024 + tcj * 512 : half * 1024 + (tcj + 1) * 512],
                            start=True,
                            stop=True,
                        )
                        nc.vector.tensor_copy(rbs2[:, ts(tcj, 512)], rb[:])
                    cv2n = psc.tile([64, 1024], dt.float32, tag="ch", bufs=2)
                    nc.vector.tensor_tensor(
                        cv2n[:],
                        cv2[0:64, half * 1024 : (half + 1) * 1024],
                        rbs2[:],
                        ALU.mult,
                    )
                    nc.vector.tensor_sub(
                        diffT[hh, half * 1024 : (half + 1) * 1024],
                        cv1n[:, half * 1024 : (half + 1) * 1024],
                        cv2n[:],
                    )
                psc_cm.__exit__(None, None, None)

        pnorm_cm.__exit__(None, None, None)
        if use_mask:
            pmk_cm.__exit__(None, None, None)
        pe_cm.__exit__(None, None, None)
        pcatt_cm.__exit__(None, None, None)
        pqkv_cm.__exit__(None, None, None)

        # ---- stage D: partial output projection ----
        with nc.named_scope("proj"):
            pout_cm = tc.tile_pool(name="pout", bufs=2)
            pout = pout_cm.__enter__()
            psd_cm = tc.tile_pool(name="psD", bufs=4, space="PSUM")
            psd = psd_cm.__enter__()
            for tb in range(NSB):
                pd = psd.tile([128, C], dt.float32, tag="pd")
                nc.tensor.matmul(
                    pd[:], diffT[:, ts(tb, 128)], projT_s[:], start=True, stop=True
                )
                o = pout.tile([128, C], dt.float32, tag="po")
                nc.vector.tensor_copy(o[:], pd[:])
                nc.sync.dma_start(out_d[ts(tb, 128), :], o[:])
            psd_cm.__exit__(None, None, None)
            pout_cm.__exit__(None, None, None)

        for cm in [pdiff_cm, pvaug_cm, pconst_cm]:
            cm.__exit__(None, None, None)

    nc.compile()
    return nc


def _get_kernel(use_mask):
    if use_mask not in _kernels:
        _kernels[use_mask] = _build(use_mask)
    return _kernels[use_mask]


def _shard_inputs(x, y, attn_x_mask, qkv_w, qkv_b, proj_w, use_mask):
    in_maps = []
    mask01T = None
    if use_mask:
        mask01T = np.ascontiguousarray(
            np.asarray(attn_x_mask)[0, 0].T.astype(np.float32)
        )
    for core in range(NC):
        b, hp = divmod(core, 4)
        h0, h1 = 2 * hp, 2 * hp + 1
        hs = np.r_[h0 * D : (h0 + 1) * D, h1 * D : (h1 + 1) * D]
        w_sel = qkv_w[np.r_[hs, C + hs, 2 * C + hs], :]
        m = {
            "xT": np.ascontiguousarray(x[b].T),
            "yT": np.ascontiguousarray(y[b].T),
            "wT": np.ascontiguousarray(w_sel.T),
            "bias_x": np.stack(
                [qkv_b[hs] / 8, qkv_b[C + hs], qkv_b[2 * C + hs]]
            ).reshape(3, 128, 1),
            "bias_y": np.stack(
                [qkv_b[hs] / 64, qkv_b[C + hs], qkv_b[2 * C + hs]]
            ).reshape(3, 128, 1),
            "projT": np.ascontiguousarray(proj_w.T[hs, :]),
        }
        if use_mask:
            m["mask01T"] = mask01T
        in_maps.append({k: np.ascontiguousarray(v, np.float32) for k, v in m.items()})
    return in_maps


def _run(x, y, attn_x_mask, qkv_w, qkv_b, proj_w, proj_b, profile=False):
    from concourse.bass_utils import run_bass_kernel_spmd

    x = np.asarray(x, np.float32)
    y = np.asarray(y, np.float32)
    qkv_w = np.asarray(qkv_w, np.float32)
    qkv_b = np.asarray(qkv_b, np.float32)
    proj_w = np.asarray(proj_w, np.float32)
    proj_b = np.asarray(proj_b, np.float32)
    mask = np.asarray(attn_x_mask)
    use_mask = not bool(mask.all())

    if profile:
        _install_ntff_hook()
    nc = _get_kernel(use_mask)
    in_maps = _shard_inputs(x, y, mask, qkv_w, qkv_b, proj_w, use_mask)
    res = run_bass_kernel_spmd(nc, in_maps, list(range(NC)), trace=profile)

    out = np.zeros((B, T, C), np.float64)
    for core in range(NC):
        b = core // 4
        out[b] += res.results[core]["out_partial"].astype(np.float64)
    out += proj_b.astype(np.float64)
    return out.astype(np.float32), res


def kernel(x, y, attn_x_mask, qkv_w, qkv_b, proj_w, proj_b):
    out, _ = _run(x, y, attn_x_mask, qkv_w, qkv_b, proj_w, proj_b, profile=False)
    return out


def kernel_profiled(x, y, attn_x_mask, qkv_w, qkv_b, proj_w, proj_b):
    out, res = _run(x, y, attn_x_mask, qkv_w, qkv_b, proj_w, proj_b, profile=True)
    return out, res


# revision 5
# speedup vs baseline: 1.0737x; 1.0737x over previous
"""Trainium2 Bass kernel for nn_CrossAttentionPro (chained cross-attention).

Sharding: 8 cores = data-parallel over B (2) x head-parallel (4 head-pairs).
Each core computes, for one batch b and heads (2*hp, 2*hp+1):
  - shared qkv projection of x and y restricted to its heads (column-sharded
    qkv_w), with attention scales folded into q biases/scales
  - catt_x2yT[m,t] = k_y q_x^T (pre-transposed layout), catt_y2x[m,s]
    (carries the extra chained 1/8 via q_y scale 1/64)
  - softmax-free-of-max attention both stages via exp + ones-column matmul
    (appended ones column of v gives the softmax denominator for free)
  - chainedT[s,t] accumulated per 128-row s-block, exp'd, and consumed
    immediately by the cval2 matmul (flash-style, never hits HBM)
  - partial projection out_partial[t, :] = diffT.T @ proj_w.T[c_slice]
Host sums the 4 head-pair partials per batch and adds proj_b.

All heavy matmuls run in float32r (full PE rate, ~1.5e-4 matmul rel err).
"""

import math
import numpy as np

B, T, MM, C, H = 2, 2048, 1024, 512, 8
D = 64
NC = 8
NMB = MM // 128  # 8 m-blocks
NSB = T // 128  # 16 s-blocks
NTC = T // 512  # 4 t-chunks of 512
_kernels = {}


def _install_ntff_hook():
    """Bridge antenv.axon_hooks for NTFF profiling (missing in this image)."""
    import contextlib, ctypes, sys, types

    if "antenv.axon_hooks" in sys.modules:
        return
    try:
        import antenv
    except ImportError:
        return

    def _make_hook():
        try:
            lib = ctypes.CDLL("/opt/axon/libaxon_pjrt.so")
        except OSError:
            return None
        if not hasattr(lib, "axon_start_nrt_profile"):
            return None
        lib.axon_start_nrt_profile.argtypes = [
            ctypes.POINTER(ctypes.c_int64),
            ctypes.c_size_t,
        ]
        lib.axon_start_nrt_profile.restype = ctypes.c_int64
        lib.axon_stop_nrt_profile.argtypes = [ctypes.c_char_p]
        lib.axon_stop_nrt_profile.restype = ctypes.c_int64

        @contextlib.contextmanager
        def _hook(output_dir, device_ids):
            import jax

            jax.devices()
            if device_ids:
                ids = (ctypes.c_int64 * len(device_ids))(*device_ids)
                rc = lib.axon_start_nrt_profile(ids, len(device_ids))
            else:
                rc = lib.axon_start_nrt_profile(None, 0)
            if rc != 0:
                raise RuntimeError(f"axon_start_nrt_profile rc={rc}")
            try:
                yield
            finally:
                n = lib.axon_stop_nrt_profile(str(output_dir).encode())
                if n < 0:
                    raise RuntimeError(f"axon_stop_nrt_profile rc={n}")

        return _hook

    m = types.ModuleType("antenv.axon_hooks")
    m._hook = _make_hook()
    m.get_axon_ntff_profile_hook = lambda: m._hook
    m.set_axon_ntff_profile_hook = lambda h: setattr(m, "_hook", h)
    sys.modules["antenv.axon_hooks"] = m
    antenv.axon_hooks = m


def _build(use_mask):
    import concourse.bass as bass
    import concourse.mybir as mybir
    import concourse.tile as tile
    from concourse import bacc
    from concourse.bass import ts
    from concourse.masks import make_identity

    dt = mybir.dt
    AF = mybir.ActivationFunctionType
    ALU = mybir.AluOpType

    nc = bacc.Bacc("TRN2", target_bir_lowering=False, debug=False, num_devices=NC)
    xT_d = nc.dram_tensor("xT", [C, T], dt.float32r, kind="ExternalInput").ap()
    yT_d = nc.dram_tensor("yT", [C, MM], dt.float32r, kind="ExternalInput").ap()
    wT_d = nc.dram_tensor("wT", [C, 384], dt.float32r, kind="ExternalInput").ap()
    bx_d = nc.dram_tensor("bias_x", [3, 128, 1], dt.float32, kind="ExternalInput").ap()
    by_d = nc.dram_tensor("bias_y", [3, 128, 1], dt.float32, kind="ExternalInput").ap()
    pw_d = nc.dram_tensor("projT", [128, C], dt.float32r, kind="ExternalInput").ap()
    if use_mask:
        mk_d = nc.dram_tensor("mask01T", [T, T], dt.float32, kind="ExternalInput").ap()
    out_d = nc.dram_tensor("out_partial", [T, C], dt.float32, kind="ExternalOutput").ap()

    with tile.TileContext(nc) as tc:
        # pool stack (LIFO): pconst > pvaug > pdiff > pqkv > {pin,pvt} /
        # {pcatt,pE,(pmk),pnorm} / {pout}
        pconst_cm = tc.tile_pool(name="pconst", bufs=1)
        pconst = pconst_cm.__enter__()
        pvaug_cm = tc.tile_pool(name="pvaug", bufs=1)
        pvaug = pvaug_cm.__enter__()
        pdiff_cm = tc.tile_pool(name="pdiff", bufs=1)
        pdiff = pdiff_cm.__enter__()
        pqkv_cm = tc.tile_pool(name="pqkv", bufs=1)
        pqkv = pqkv_cm.__enter__()
        pin_cm = tc.tile_pool(name="pin", bufs=1)
        pin = pin_cm.__enter__()
        pvt_cm = tc.tile_pool(name="pvt", bufs=1)
        pvt = pvt_cm.__enter__()

        # ---- constants ----
        ident = pconst.tile([128, 128], dt.float32, tag="ident")
        make_identity(nc, ident[:])
        ones16 = pconst.tile([128, 16], dt.float32, tag="ones16")
        nc.vector.memset(ones16[:], 1.0)
        biases = {}
        for i, nm in enumerate(["q", "k", "v"]):
            bx = pconst.tile([128, 1], dt.float32, tag=f"bx_{nm}")
            nc.sync.dma_start(bx[:], bx_d[i])
            by = pconst.tile([128, 1], dt.float32, tag=f"by_{nm}")
            nc.sync.dma_start(by[:], by_d[i])
            biases[("x", nm)] = bx
            biases[("y", nm)] = by
        projT_s = pconst.tile([128, C], dt.float32r, tag="projT")
        nc.sync.dma_start(projT_s[:], pw_d[:])

        # ---- stage A: load inputs, qkv projections, v transposes ----
        with nc.named_scope("stageA"):
            xT = [pin.tile([128, T], dt.float32r, tag=f"xT{i}", name=f"xT{i}") for i in range(4)]
            yT = [pin.tile([128, MM], dt.float32r, tag=f"yT{i}", name=f"yT{i}") for i in range(4)]
            wT = [pin.tile([128, 384], dt.float32r, tag=f"wT{i}", name=f"wT{i}") for i in range(4)]
            for i in range(4):
                nc.sync.dma_start(xT[i][:], xT_d[ts(i, 128), :])
                nc.sync.dma_start(yT[i][:], yT_d[ts(i, 128), :])
                nc.sync.dma_start(wT[i][:], wT_d[ts(i, 128), :])

            qx = pqkv.tile([128, T], dt.float32r, tag="qx")
            kx = pqkv.tile([128, T], dt.float32r, tag="kx")
            qy = pqkv.tile([128, MM], dt.float32r, tag="qy")
            ky = pqkv.tile([128, MM], dt.float32r, tag="ky")
            vxT = pvt.tile([128, T], dt.float32, tag="vxT")
            vyT = pvt.tile([128, MM], dt.float32, tag="vyT")

            projs = [
                (qx, xT, T, 0, ("x", "q"), 1.0 / 8),
                (kx, xT, T, 128, ("x", "k"), 1.0),
                (vxT, xT, T, 256, ("x", "v"), 1.0),
                (qy, yT, MM, 0, ("y", "q"), 1.0 / 64),
                (ky, yT, MM, 128, ("y", "k"), 1.0),
                (vyT, yT, MM, 256, ("y", "v"), 1.0),
            ]
            psa_cm = tc.tile_pool(name="psA", bufs=2, space="PSUM")
            psa = psa_cm.__enter__()
            for out_t, src, n_t, wcol, bkey, scale in projs:
                ps = psa.tile([128, n_t], dt.float32, tag="pa")
                for c in range(4):
                    for tcj in range(n_t // 512):
                        nc.tensor.matmul(
                            ps[:, ts(tcj, 512)],
                            wT[c][:, wcol : wcol + 128],
                            src[c][:, ts(tcj, 512)],
                            start=(c == 0),
                            stop=(c == 3),
                        )
                nc.scalar.activation(
                    out_t[:], ps[:], AF.Identity, bias=biases[bkey][:], scale=scale
                )
            psa_cm.__exit__(None, None, None)

            # v transposes -> v_aug tensors (both heads), ones col appended
            vx_aug = [
                pvaug.tile([128, 65 * NSB], dt.float32r, tag=f"vx_aug{h}", name=f"vx_aug{h}")
                for h in range(2)
            ]
            vy_aug = [
                pvaug.tile([128, 65 * NMB], dt.float32r, tag=f"vy_aug{h}", name=f"vy_aug{h}")
                for h in range(2)
            ]
            pst_cm = tc.tile_pool(name="psT", bufs=4, space="PSUM")
            pst = pst_cm.__enter__()
            for src, aug, nblk in [(vxT, vx_aug, NSB), (vyT, vy_aug, NMB)]:
                for i in range(nblk):
                    tp = pst.tile([128, 128], dt.float32, tag="tp")
                    nc.tensor.transpose(tp[:], src[:, ts(i, 128)], ident[:])
                    for h in range(2):
                        nc.vector.tensor_copy(
                            aug[h][:, 65 * i : 65 * i + 64], tp[:, ts(h, 64)]
                        )
            pst_cm.__exit__(None, None, None)
            for aug, nblk in [(vx_aug, NSB), (vy_aug, NMB)]:
                for h in range(2):
                    nc.vector.tensor_copy(
                        aug[h][:, 64 : 65 * nblk : 65], ones16[:, 0:nblk]
                    )
        pvt_cm.__exit__(None, None, None)
        pin_cm.__exit__(None, None, None)

        pcatt_cm = tc.tile_pool(name="pcatt", bufs=1)
        pcatt = pcatt_cm.__enter__()
        pe_cm = tc.tile_pool(name="pE", bufs=1 if use_mask else 2)
        pE = pe_cm.__enter__()
        if use_mask:
            pmk_cm = tc.tile_pool(name="pmk", bufs=2)
            pmk = pmk_cm.__enter__()
        pnorm_cm = tc.tile_pool(name="pnorm", bufs=1)
        pnorm = pnorm_cm.__enter__()

        diffT = pdiff.tile([128, T], dt.float32r, tag="diffT")

        for h in range(2):
            hh = slice(64 * h, 64 * h + 64)
            cxT = pcatt.tile([128, T * NMB], dt.float32r, tag="cxT")
            cyx = pcatt.tile([128, T * NMB], dt.float32r, tag="cyx")

            # ---- stage B + C1: catt tiles, E1, cval1 accumulation ----
            with nc.named_scope(f"B{h}"):
                psb_cm = tc.tile_pool(name=f"psB{h}", bufs=1, space="PSUM")
                psb = psb_cm.__enter__()
                cv1 = psb.tile([65, T], dt.float32, tag="cv")
                for mb in range(NMB):
                    for half in range(2):
                        pb = psb.tile([128, 1024], dt.float32, tag="pb", bufs=2)
                        for tcj in range(2):
                            nc.tensor.matmul(
                                pb[:, ts(tcj, 512)],
                                ky[hh, ts(mb, 128)],
                                qx[hh, half * 1024 + tcj * 512 : half * 1024 + (tcj + 1) * 512],
                                start=True,
                                stop=True,
                                tile_position=(64 * h, 0),
                            )
                        nc.vector.tensor_copy(
                            cxT[:, mb * T + half * 1024 : mb * T + (half + 1) * 1024],
                            pb[:],
                        )
                        e1 = pE.tile([128, 1024], dt.float32r, tag="E")
                        nc.scalar.activation(e1[:], pb[:], AF.Exp)
                        for tcj in range(2):
                            nc.tensor.matmul(
                                cv1[:, half * 1024 + tcj * 512 : half * 1024 + (tcj + 1) * 512],
                                vy_aug[h][:, 65 * mb : 65 * mb + 65],
                                e1[:, ts(tcj, 512)],
                                start=(mb == 0),
                                stop=(mb == NMB - 1),
                            )
                    for half in range(2):
                        pb2 = psb.tile([128, 1024], dt.float32, tag="pb", bufs=2)
                        for tcj in range(2):
                            nc.tensor.matmul(
                                pb2[:, ts(tcj, 512)],
                                qy[hh, ts(mb, 128)],
                                kx[hh, half * 1024 + tcj * 512 : half * 1024 + (tcj + 1) * 512],
                                start=True,
                                stop=True,
                                tile_position=(64 * h, 0),
                            )
                        nc.vector.tensor_copy(
                            cyx[:, mb * T + half * 1024 : mb * T + (half + 1) * 1024],
                            pb2[:],
                        )

                # cval1 normalization: r1 = 1/sumexp; cv1n = cv1 * bcast(r1)
                r1 = pnorm.tile([1, T], dt.float32, tag="r")
                rbs = pnorm.tile([64, T], dt.float32, tag="rbs")
                nc.scalar.copy(rbs[0:1, :], cv1[64:65, :])
                nc.vector.reciprocal_approx_fast(r1[:], rbs[0:1, :])
                nc.gpsimd.partition_broadcast(rbs[:], r1[:])
                cv1n = pnorm.tile([64, T], dt.float32, tag="cv1n")
                nc.vector.tensor_tensor(cv1n[:], cv1[0:64, :], rbs[:], ALU.mult)
                psb_cm.__exit__(None, None, None)

            # ---- stage C2: chained scores + cval2, flash-style ----
            with nc.named_scope(f"C{h}"):
                psc_cm = tc.tile_pool(name=f"psC{h}", bufs=1, space="PSUM")
                psc = psc_cm.__enter__()
                cv2 = psc.tile([65, T], dt.float32, tag="cv")
                for sbi in range(NSB):
                    for half in range(2):
                        ch = psc.tile([128, 1024], dt.float32, tag="ch", bufs=2)
                        for mb in range(NMB):
                            for tcj in range(2):
                                nc.tensor.matmul(
                                    ch[:, ts(tcj, 512)],
                                    cyx[:, mb * T + sbi * 128 : mb * T + (sbi + 1) * 128],
                                    cxT[:, mb * T + half * 1024 + tcj * 512 : mb * T + half * 1024 + (tcj + 1) * 512],
                                    start=(mb == 0),
                                    stop=(mb == NMB - 1),
                                )
                        e2 = pE.tile([128, 1024], dt.float32r, tag="E")
                        nc.scalar.activation(e2[:], ch[:], AF.Exp)
                        if use_mask:
                            for tcj in range(2):
                                mk = pmk.tile([128, 512], dt.float32, tag="mk")
                                nc.sync.dma_start(
                                    mk[:],
                                    mk_d[
                                        ts(sbi, 128),
                                        half * 1024 + tcj * 512 : half * 1024 + (tcj + 1) * 512,
                                    ],
                                )
                                nc.vector.tensor_tensor(
                                    e2[:, ts(tcj, 512)],
                                    e2[:, ts(tcj, 512)],
                                    mk[:],
                                    ALU.mult,
                                )
                        for tcj in range(2):
                            nc.tensor.matmul(
                                cv2[:, half * 1024 + tcj * 512 : half * 1024 + (tcj + 1) * 512],
                                vx_aug[h][:, 65 * sbi : 65 * sbi + 65],
                                e2[:, ts(tcj, 512)],
                                start=(sbi == 0),
                                stop=(sbi == NSB - 1),
                            )

                # cval2 normalization + diff into diffT rows of this head
                r2 = pnorm.tile([1, T], dt.float32, tag="r")
                rbs2 = pnorm.tile([64, T], dt.float32, tag="rbs")
                nc.scalar.copy(rbs2[0:1, :], cv2[64:65, :])
                nc.vector.reciprocal_approx_fast(r2[:], rbs2[0:1, :])
                nc.gpsimd.partition_broadcast(rbs2[:], r2[:])
                for half in range(2):
                    cv2n = psc.tile([64, 1024], dt.float32, tag="ch", bufs=2)
                    nc.vector.tensor_tensor(
                        cv2n[:],
                        cv2[0:64, half * 1024 : (half + 1) * 1024],
                        rbs2[:, half * 1024 : (half + 1) * 1024],
                        ALU.mult,
                    )
                    nc.vector.tensor_sub(
                        diffT[hh, half * 1024 : (half + 1) * 1024],
                        cv1n[:, half * 1024 : (half + 1) * 1024],
                        cv2n[:],
                    )
                psc_cm.__exit__(None, None, None)

        pnorm_cm.__exit__(None, None, None)
        if use_mask:
            pmk_cm.__exit__(None, None, None)
        pe_cm.__exit__(None, None, None)
        pcatt_cm.__exit__(None, None, None)
        pqkv_cm.__exit__(None, None, None)

        # ---- stage D: partial output projection ----
        with nc.named_scope("proj"):
            pout_cm = tc.tile_pool(name="pout", bufs=2)
            pout = pout_cm.__enter__()
            psd_cm = tc.tile_pool(name="psD", bufs=4, space="PSUM")
            psd = psd_cm.__enter__()
            for tb in range(NSB):
                pd = psd.tile([128, C], dt.float32, tag="pd")
                nc.tensor.matmul(
                    pd[:], diffT[:, ts(tb, 128)], projT_s[:], start=True, stop=True
                )
                o = pout.tile([128, C], dt.float32, tag="po")
                nc.vector.tensor_copy(o[:], pd[:])
                nc.sync.dma_start(out_d[ts(tb, 128), :], o[:])
            psd_cm.__exit__(None, None, None)
            pout_cm.__exit__(None, None, None)

        for cm in [pdiff_cm, pvaug_cm, pconst_cm]:
            cm.__exit__(None, None, None)

    nc.compile()
    return nc


def _get_kernel(use_mask):
    if use_mask not in _kernels:
        _kernels[use_mask] = _build(use_mask)
    return _kernels[use_mask]


def _shard_inputs(x, y, attn_x_mask, qkv_w, qkv_b, proj_w, use_mask):
    in_maps = []
    mask01T = None
    if use_mask:
        mask01T = np.ascontiguousarray(
            np.asarray(attn_x_mask)[0, 0].T.astype(np.float32)
        )
    for core in range(NC):
        b, hp = divmod(core, 4)
        h0, h1 = 2 * hp, 2 * hp + 1
        hs = np.r_[h0 * D : (h0 + 1) * D, h1 * D : (h1 + 1) * D]
        w_sel = qkv_w[np.r_[hs, C + hs, 2 * C + hs], :]
        m = {
            "xT": np.ascontiguousarray(x[b].T),
            "yT": np.ascontiguousarray(y[b].T),
            "wT": np.ascontiguousarray(w_sel.T),
            "bias_x": np.stack(
                [qkv_b[hs] / 8, qkv_b[C + hs], qkv_b[2 * C + hs]]
            ).reshape(3, 128, 1),
            "bias_y": np.stack(
                [qkv_b[hs] / 64, qkv_b[C + hs], qkv_b[2 * C + hs]]
            ).reshape(3, 128, 1),
            "projT": np.ascontiguousarray(proj_w.T[hs, :]),
        }
        if use_mask:
            m["mask01T"] = mask01T
        in_maps.append({k: np.ascontiguousarray(v, np.float32) for k, v in m.items()})
    return in_maps


def _run(x, y, attn_x_mask, qkv_w, qkv_b, proj_w, proj_b, profile=False):
    from concourse.bass_utils import run_bass_kernel_spmd

    x = np.asarray(x, np.float32)
    y = np.asarray(y, np.float32)
    qkv_w = np.asarray(qkv_w, np.float32)
    qkv_b = np.asarray(qkv_b, np.float32)
    proj_w = np.asarray(proj_w, np.float32)
    proj_b = np.asarray(proj_b, np.float32)
    mask = np.asarray(attn_x_mask)
    use_mask = not bool(mask.all())

    if profile:
        _install_ntff_hook()
    nc = _get_kernel(use_mask)
    in_maps = _shard_inputs(x, y, mask, qkv_w, qkv_b, proj_w, use_mask)
    res = run_bass_kernel_spmd(nc, in_maps, list(range(NC)), trace=profile)

    out = np.zeros((B, T, C), np.float64)
    for core in range(NC):
        b = core // 4
        out[b] += res.results[core]["out_partial"].astype(np.float64)
    out += proj_b.astype(np.float64)
    return out.astype(np.float32), res


def kernel(x, y, attn_x_mask, qkv_w, qkv_b, proj_w, proj_b):
    out, _ = _run(x, y, attn_x_mask, qkv_w, qkv_b, proj_w, proj_b, profile=False)
    return out


def kernel_profiled(x, y, attn_x_mask, qkv_w, qkv_b, proj_w, proj_b):
    out, res = _run(x, y, attn_x_mask, qkv_w, qkv_b, proj_w, proj_b, profile=True)
    return out, res


# revision 6
# speedup vs baseline: 1.6105x; 1.5000x over previous
"""Trainium2 Bass kernel for nn_CrossAttentionPro (chained cross-attention).

Sharding: 8 cores = data-parallel over B (2) x head-parallel (4 head-pairs).
Each core computes, for one batch b and heads (2*hp, 2*hp+1):
  - shared qkv projection of x and y restricted to its heads (column-sharded
    qkv_w), with attention scales folded into q biases/scales
  - catt_x2yT[m,t] = k_y q_x^T (pre-transposed layout), catt_y2x[m,s]
    (carries the extra chained 1/8 via q_y scale 1/64)
  - softmax-free-of-max attention both stages via exp + ones-column matmul
    (appended ones column of v gives the softmax denominator for free)
  - chainedT[s,t] accumulated per 128-row s-block, exp'd, and consumed
    immediately by the cval2 matmul (flash-style, never hits HBM)
  - partial projection out_partial[t, :] = diffT.T @ proj_w.T[c_slice]
Host sums the 4 head-pair partials per batch and adds proj_b.

All heavy matmuls run in float32r (full PE rate, ~1.5e-4 matmul rel err).
"""

import math
import numpy as np

B, T, MM, C, H = 2, 2048, 1024, 512, 8
D = 64
NC = 8
NMB = MM // 128  # 8 m-blocks
NSB = T // 128  # 16 s-blocks
NTC = T // 512  # 4 t-chunks of 512
_kernels = {}


def _install_ntff_hook():
    """Bridge antenv.axon_hooks for NTFF profiling (missing in this image)."""
    import contextlib, ctypes, sys, types

    if "antenv.axon_hooks" in sys.modules:
        return
    try:
        import antenv
    except ImportError:
        return

    def _make_hook():
        try:
            lib = ctypes.CDLL("/opt/axon/libaxon_pjrt.so")
        except OSError:
            return None
        if not hasattr(lib, "axon_start_nrt_profile"):
            return None
        lib.axon_start_nrt_profile.argtypes = [
            ctypes.POINTER(ctypes.c_int64),
            ctypes.c_size_t,
        ]
        lib.axon_start_nrt_profile.restype = ctypes.c_int64
        lib.axon_stop_nrt_profile.argtypes = [ctypes.c_char_p]
        lib.axon_stop_nrt_profile.restype = ctypes.c_int64

        @contextlib.contextmanager
        def _hook(output_dir, device_ids):
            import jax

            jax.devices()
            if device_ids:
                ids = (ctypes.c_int64 * len(device_ids))(*device_ids)
                rc = lib.axon_start_nrt_profile(ids, len(device_ids))
            else:
                rc = lib.axon_start_nrt_profile(None, 0)
            if rc != 0:
                raise RuntimeError(f"axon_start_nrt_profile rc={rc}")
            try:
                yield
            finally:
                n = lib.axon_stop_nrt_profile(str(output_dir).encode())
                if n < 0:
                    raise RuntimeError(f"axon_stop_nrt_profile rc={n}")

        return _hook

    m = types.ModuleType("antenv.axon_hooks")
    m._hook = _make_hook()
    m.get_axon_ntff_profile_hook = lambda: m._hook
    m.set_axon_ntff_profile_hook = lambda h: setattr(m, "_hook", h)
    sys.modules["antenv.axon_hooks"] = m
    antenv.axon_hooks = m


def _build(use_mask):
    import concourse.bass as bass
    import concourse.mybir as mybir
    import concourse.tile as tile
    from concourse import bacc
    from concourse.bass import ts
    from concourse.masks import make_identity

    dt = mybir.dt
    AF = mybir.ActivationFunctionType
    ALU = mybir.AluOpType

    nc = bacc.Bacc("TRN2", target_bir_lowering=False, debug=False, num_devices=NC)
    xT_d = nc.dram_tensor("xT", [C, T], dt.float32r, kind="ExternalInput").ap()
    yT_d = nc.dram_tensor("yT", [C, MM], dt.float32r, kind="ExternalInput").ap()
    wT_d = nc.dram_tensor("wT", [C, 384], dt.float32r, kind="ExternalInput").ap()
    bx_d = nc.dram_tensor("bias_x", [3, 128, 1], dt.float32, kind="ExternalInput").ap()
    by_d = nc.dram_tensor("bias_y", [3, 128, 1], dt.float32, kind="ExternalInput").ap()
    pw_d = nc.dram_tensor("projT", [128, C], dt.float32r, kind="ExternalInput").ap()
    if use_mask:
        mk_d = nc.dram_tensor("mask01T", [T, T], dt.float32, kind="ExternalInput").ap()
    out_d = nc.dram_tensor("out_partial", [T, C], dt.float32, kind="ExternalOutput").ap()

    with tile.TileContext(nc) as tc:
        pconst_cm = tc.tile_pool(name="pconst", bufs=1)
        pconst = pconst_cm.__enter__()
        pbig_cm = tc.tile_pool(name="pbig", bufs=1)
        pbig = pbig_cm.__enter__()
        pe_cm = tc.tile_pool(name="pE", bufs=4)
        pE = pe_cm.__enter__()
        if use_mask:
            pmk_cm = tc.tile_pool(name="pmk", bufs=2)
            pmk = pmk_cm.__enter__()
        pout_cm = tc.tile_pool(name="pout", bufs=2)
        pout = pout_cm.__enter__()
        pin_cm = tc.tile_pool(name="pin", bufs=1)
        pin = pin_cm.__enter__()

        # ---- constants ----
        ident = pconst.tile([128, 128], dt.float32, tag="ident")
        make_identity(nc, ident[:])
        ones16 = pconst.tile([128, 16], dt.float32, tag="ones16")
        nc.vector.memset(ones16[:], 1.0)
        biases = {}
        for i, nm in enumerate(["q", "k", "v"]):
            bx = pconst.tile([128, 1], dt.float32, tag=f"bx_{nm}")
            nc.sync.dma_start(bx[:], bx_d[i])
            by = pconst.tile([128, 1], dt.float32, tag=f"by_{nm}")
            nc.sync.dma_start(by[:], by_d[i])
            biases[("x", nm)] = bx
            biases[("y", nm)] = by
        projT_s = pconst.tile([128, C], dt.float32r, tag="projT")
        nc.sync.dma_start(projT_s[:], pw_d[:])

        # ---- stage A: loads, projections, transposes, G, W_x ----
        with nc.named_scope("stageA"):
            xT = [pin.tile([128, T], dt.float32r, tag=f"xT{i}", name=f"xT{i}") for i in range(4)]
            yT = [pin.tile([128, MM], dt.float32r, tag=f"yT{i}", name=f"yT{i}") for i in range(4)]
            wT = [pin.tile([128, 384], dt.float32r, tag=f"wT{i}", name=f"wT{i}") for i in range(4)]
            for i in range(4):
                nc.sync.dma_start(xT[i][:], xT_d[ts(i, 128), :])
                nc.sync.dma_start(yT[i][:], yT_d[ts(i, 128), :])
                nc.sync.dma_start(wT[i][:], wT_d[ts(i, 128), :])

            qx = pbig.tile([128, T], dt.float32r, tag="qx")
            kx = pbig.tile([128, T], dt.float32r, tag="kx")
            qy = pbig.tile([128, MM], dt.float32r, tag="qy")
            ky = pbig.tile([128, MM], dt.float32r, tag="ky")
            vxT = pin.tile([128, T], dt.float32, tag="vxT")
            vyT = pin.tile([128, MM], dt.float32, tag="vyT")

            projs = [
                (qx, xT, T, 0, ("x", "q"), 1.0 / 8),
                (kx, xT, T, 128, ("x", "k"), 1.0),
                (vxT, xT, T, 256, ("x", "v"), 1.0),
                (qy, yT, MM, 0, ("y", "q"), 1.0 / 64),
                (ky, yT, MM, 128, ("y", "k"), 1.0),
                (vyT, yT, MM, 256, ("y", "v"), 1.0),
            ]
            psa_cm = tc.tile_pool(name="psA", bufs=2, space="PSUM")
            psa = psa_cm.__enter__()
            for out_t, src, n_t, wcol, bkey, scale in projs:
                ps = psa.tile([128, n_t], dt.float32, tag="pa")
                for c in range(4):
                    for tcj in range(n_t // 512):
                        nc.tensor.matmul(
                            ps[:, ts(tcj, 512)],
                            wT[c][:, wcol : wcol + 128],
                            src[c][:, ts(tcj, 512)],
                            start=(c == 0),
                            stop=(c == 3),
                        )
                nc.scalar.activation(
                    out_t[:], ps[:], AF.Identity, bias=biases[bkey][:], scale=scale
                )
            psa_cm.__exit__(None, None, None)

            # transposes: v_aug for both heads; token-major qy/ky for G
            vx_aug = [pbig.tile([128, 65 * NSB], dt.float32r, tag=f"vx_aug{h}", name=f"vx_aug{h}") for h in range(2)]
            vy_aug = [pbig.tile([128, 65 * NMB], dt.float32r, tag=f"vy_aug{h}", name=f"vy_aug{h}") for h in range(2)]
            qy_tok = pbig.tile([128, MM], dt.float32r, tag="qy_tok")
            ky_tok = pbig.tile([128, MM], dt.float32r, tag="ky_tok")

            pst_cm = tc.tile_pool(name="psT", bufs=4, space="PSUM")
            pst = pst_cm.__enter__()
            for src, aug, nblk in [(vxT, vx_aug, NSB), (vyT, vy_aug, NMB)]:
                for i in range(nblk):
                    tp = pst.tile([128, 128], dt.float32, tag="tp")
                    nc.tensor.transpose(tp[:], src[:, ts(i, 128)], ident[:])
                    for h in range(2):
                        nc.vector.tensor_copy(
                            aug[h][:, 65 * i : 65 * i + 64], tp[:, ts(h, 64)]
                        )
            for aug, nblk in [(vx_aug, NSB), (vy_aug, NMB)]:
                for h in range(2):
                    nc.vector.tensor_copy(
                        aug[h][:, 64 : 65 * nblk : 65], ones16[:, 0:nblk]
                    )
            for src, dst in [(qy, qy_tok), (ky, ky_tok)]:
                for i in range(NMB):
                    tp = pst.tile([128, 128], dt.float32, tag="tp")
                    nc.tensor.transpose(tp[:], src[:, ts(i, 128)].bitcast(dt.float32), ident[:])
                    nc.vector.tensor_copy(dst[:, ts(i, 128)], tp[:])

            # G^T (block-diagonal over the 2 heads): GT = Qy_tok^T @ Ky_tok
            gt_ps = pst.tile([128, 128], dt.float32, tag="gt")
            for mb in range(NMB):
                nc.tensor.matmul(
                    gt_ps[:],
                    qy_tok[:, ts(mb, 128)],
                    ky_tok[:, ts(mb, 128)],
                    start=(mb == 0),
                    stop=(mb == NMB - 1),
                )
            gt_z = pin.tile([128, 128], dt.float32, tag="gt_z")
            nc.vector.memset(gt_z[:], 0.0)
            gt_s = pbig.tile([128, 128], dt.float32r, tag="gt_s")
            nc.vector.tensor_copy(gt_s[:], gt_z[:])
            nc.vector.tensor_copy(gt_s[0:64, 0:64], gt_ps[0:64, 0:64])
            nc.vector.tensor_copy(gt_s[64:128, 64:128], gt_ps[64:128, 64:128])
            pst_cm.__exit__(None, None, None)

            # W_xT[d, s] = (G @ kxT) for both heads at once (block-diag GT)
            wxT = pbig.tile([128, T], dt.float32r, tag="wxT")
            psw_cm = tc.tile_pool(name="psW", bufs=1, space="PSUM")
            psw = psw_cm.__enter__()
            wx_ps = psw.tile([128, T], dt.float32, tag="wx")
            for tcj in range(NTC):
                nc.tensor.matmul(
                    wx_ps[:, ts(tcj, 512)],
                    gt_s[:],
                    kx[:, ts(tcj, 512)],
                    start=True,
                    stop=True,
                )
            nc.vector.tensor_copy(wxT[:], wx_ps[:])
            psw_cm.__exit__(None, None, None)
        pin_cm.__exit__(None, None, None)

        pnorm_cm = tc.tile_pool(name="pnorm", bufs=1)
        pnorm = pnorm_cm.__enter__()
        pdiff_cm = tc.tile_pool(name="pdiff", bufs=1)
        pdiff = pdiff_cm.__enter__()
        diffT = pdiff.tile([128, T], dt.float32r, tag="diffT")

        for h in range(2):
            hh = slice(64 * h, 64 * h + 64)
            tpos = (64 * h, 0) if h else None

            # ---- stage B: x2y scores -> exp -> cval1 (flash-style) ----
            with nc.named_scope(f"B{h}"):
                psb_cm = tc.tile_pool(name=f"psB{h}", bufs=1, space="PSUM")
                psb = psb_cm.__enter__()
                cv1 = psb.tile([65, T], dt.float32, tag="cv")
                for mb in range(NMB):
                    for half in range(2):
                        pb = psb.tile([128, 1024], dt.float32, tag="pb", bufs=2)
                        for tcj in range(2):
                            nc.tensor.matmul(
                                pb[:, ts(tcj, 512)],
                                ky[hh, ts(mb, 128)],
                                qx[hh, half * 1024 + tcj * 512 : half * 1024 + (tcj + 1) * 512],
                                start=True,
                                stop=True,
                                tile_position=tpos,
                            )
                        e1 = pE.tile([128, 1024], dt.float32r, tag="E")
                        nc.scalar.activation(e1[:], pb[:], AF.Exp)
                        for tcj in range(2):
                            nc.tensor.matmul(
                                cv1[:, half * 1024 + tcj * 512 : half * 1024 + (tcj + 1) * 512],
                                vy_aug[h][:, 65 * mb : 65 * mb + 65],
                                e1[:, ts(tcj, 512)],
                                start=(mb == 0),
                                stop=(mb == NMB - 1),
                            )

                r1 = pnorm.tile([1, T], dt.float32, tag="r")
                rbs = pnorm.tile([64, T], dt.float32, tag="rbs")
                nc.scalar.copy(rbs[0:1, :], cv1[64:65, :])
                nc.vector.reciprocal_approx_fast(r1[:], rbs[0:1, :])
                nc.gpsimd.partition_broadcast(rbs[:], r1[:])
                cv1n = pnorm.tile([64, T], dt.float32, tag="cv1n")
                nc.vector.tensor_tensor(cv1n[:], cv1[0:64, :], rbs[:], ALU.mult)
                psb_cm.__exit__(None, None, None)

            # ---- stage C: chained scores via W_x -> exp -> cval2 ----
            with nc.named_scope(f"C{h}"):
                psc_cm = tc.tile_pool(name=f"psC{h}", bufs=1, space="PSUM")
                psc = psc_cm.__enter__()
                cv2 = psc.tile([65, T], dt.float32, tag="cv")
                for sbi in range(NSB):
                    for half in range(2):
                        ch = psc.tile([128, 1024], dt.float32, tag="ch", bufs=2)
                        for tcj in range(2):
                            nc.tensor.matmul(
                                ch[:, ts(tcj, 512)],
                                wxT[hh, ts(sbi, 128)],
                                qx[hh, half * 1024 + tcj * 512 : half * 1024 + (tcj + 1) * 512],
                                start=True,
                                stop=True,
                                tile_position=tpos,
                            )
                        e2 = pE.tile([128, 1024], dt.float32r, tag="E")
                        nc.scalar.activation(e2[:], ch[:], AF.Exp)
                        if use_mask:
                            for tcj in range(2):
                                mk = pmk.tile([128, 512], dt.float32, tag="mk")
                                nc.sync.dma_start(
                                    mk[:],
                                    mk_d[
                                        ts(sbi, 128),
                                        half * 1024 + tcj * 512 : half * 1024 + (tcj + 1) * 512,
                                    ],
                                )
                                nc.vector.tensor_tensor(
                                    e2[:, ts(tcj, 512)],
                                    e2[:, ts(tcj, 512)],
                                    mk[:],
                                    ALU.mult,
                                )
                        for tcj in range(2):
                            nc.tensor.matmul(
                                cv2[:, half * 1024 + tcj * 512 : half * 1024 + (tcj + 1) * 512],
                                vx_aug[h][:, 65 * sbi : 65 * sbi + 65],
                                e2[:, ts(tcj, 512)],
                                start=(sbi == 0),
                                stop=(sbi == NSB - 1),
                            )

                r2 = pnorm.tile([1, T], dt.float32, tag="r")
                rbs2 = pnorm.tile([64, T], dt.float32, tag="rbs")
                nc.scalar.copy(rbs2[0:1, :], cv2[64:65, :])
                nc.vector.reciprocal_approx_fast(r2[:], rbs2[0:1, :])
                nc.gpsimd.partition_broadcast(rbs2[:], r2[:])
                cv1n_h = cv1n
                for half in range(2):
                    cv2n = psc.tile([64, 1024], dt.float32, tag="ch", bufs=2)
                    nc.vector.tensor_tensor(
                        cv2n[:],
                        cv2[0:64, half * 1024 : (half + 1) * 1024],
                        rbs2[:, half * 1024 : (half + 1) * 1024],
                        ALU.mult,
                    )
                    nc.vector.tensor_sub(
                        diffT[hh, half * 1024 : (half + 1) * 1024],
                        cv1n_h[:, half * 1024 : (half + 1) * 1024],
                        cv2n[:],
                    )
                psc_cm.__exit__(None, None, None)

        # ---- stage D: partial output projection ----
        with nc.named_scope("proj"):
            psd_cm = tc.tile_pool(name="psD", bufs=4, space="PSUM")
            psd = psd_cm.__enter__()
            for tb in range(NSB):
                pd = psd.tile([128, C], dt.float32, tag="pd")
                nc.tensor.matmul(
                    pd[:], diffT[:, ts(tb, 128)], projT_s[:], start=True, stop=True
                )
                o = pout.tile([128, C], dt.float32, tag="po")
                nc.vector.tensor_copy(o[:], pd[:])
                nc.sync.dma_start(out_d[ts(tb, 128), :], o[:])
            psd_cm.__exit__(None, None, None)

        pdiff_cm.__exit__(None, None, None)
        pnorm_cm.__exit__(None, None, None)
        pout_cm.__exit__(None, None, None)
        if use_mask:
            pmk_cm.__exit__(None, None, None)
        pe_cm.__exit__(None, None, None)
        pbig_cm.__exit__(None, None, None)
        pconst_cm.__exit__(None, None, None)

    nc.compile()
    return nc


def _get_kernel(use_mask):
    if use_mask not in _kernels:
        _kernels[use_mask] = _build(use_mask)
    return _kernels[use_mask]


def _shard_inputs(x, y, attn_x_mask, qkv_w, qkv_b, proj_w, use_mask):
    in_maps = []
    mask01T = None
    if use_mask:
        mask01T = np.ascontiguousarray(
            np.asarray(attn_x_mask)[0, 0].T.astype(np.float32)
        )
    for core in range(NC):
        b, hp = divmod(core, 4)
        h0, h1 = 2 * hp, 2 * hp + 1
        hs = np.r_[h0 * D : (h0 + 1) * D, h1 * D : (h1 + 1) * D]
        w_sel = qkv_w[np.r_[hs, C + hs, 2 * C + hs], :]
        m = {
            "xT": np.ascontiguousarray(x[b].T),
            "yT": np.ascontiguousarray(y[b].T),
            "wT": np.ascontiguousarray(w_sel.T),
            "bias_x": np.stack(
                [qkv_b[hs] / 8, qkv_b[C + hs], qkv_b[2 * C + hs]]
            ).reshape(3, 128, 1),
            "bias_y": np.stack(
                [qkv_b[hs] / 64, qkv_b[C + hs], qkv_b[2 * C + hs]]
            ).reshape(3, 128, 1),
            "projT": np.ascontiguousarray(proj_w.T[hs, :]),
        }
        if use_mask:
            m["mask01T"] = mask01T
        in_maps.append({k: np.ascontiguousarray(v, np.float32) for k, v in m.items()})
    return in_maps


def _run(x, y, attn_x_mask, qkv_w, qkv_b, proj_w, proj_b, profile=False):
    from concourse.bass_utils import run_bass_kernel_spmd

    x = np.asarray(x, np.float32)
    y = np.asarray(y, np.float32)
    qkv_w = np.asarray(qkv_w, np.float32)
    qkv_b = np.asarray(qkv_b, np.float32)
    proj_w = np.asarray(proj_w, np.float32)
    proj_b = np.asarray(proj_b, np.float32)
    mask = np.asarray(attn_x_mask)
    use_mask = not bool(mask.all())

    if profile:
        _install_ntff_hook()
    nc = _get_kernel(use_mask)
    in_maps = _shard_inputs(x, y, mask, qkv_w, qkv_b, proj_w, use_mask)
    res = run_bass_kernel_spmd(nc, in_maps, list(range(NC)), trace=profile)

    out = np.zeros((B, T, C), np.float64)
    for core in range(NC):
        b = core // 4
        out[b] += res.results[core]["out_partial"].astype(np.float64)
    out += proj_b.astype(np.float64)
    return out.astype(np.float32), res


def kernel(x, y, attn_x_mask, qkv_w, qkv_b, proj_w, proj_b):
    out, _ = _run(x, y, attn_x_mask, qkv_w, qkv_b, proj_w, proj_b, profile=False)
    return out


def kernel_profiled(x, y, attn_x_mask, qkv_w, qkv_b, proj_w, proj_b):
    out, res = _run(x, y, attn_x_mask, qkv_w, qkv_b, proj_w, proj_b, profile=True)
    return out, res
